# revision 1
# baseline (speedup 1.0000x reference)
"""GQA attention kernel for 8 trn2 NeuronCores.

Sharding: tensor-parallel over the 8 KV groups (1 group = 4 Q heads per
core, both batch elements), then an AllToAll reshards the per-core
context [256 feat, 4096 rows] into row-shards [2048 feat, 512 rows] so
the output projection runs row-parallel with no reduction.

Shapes (hardcoded): B=2, S=2048, D=2048, H=32, G=8, HD=64.
"""

import numpy as np
import concourse.bass as bass
import concourse.mybir as mybir
import concourse.tile as tile
from concourse import bacc
from concourse.bass import broadcast_tensor_aps
from concourse.bass_utils import run_bass_kernel_spmd
from concourse.masks import make_identity

N_CORES = 8
B, S, D = 2, 2048, 2048
H, G, HD = 32, 8, 64
GS = H // G                       # 4 q heads per kv group
ROWS = B * S                      # 4096 flattened (b, s) rows
RPC = ROWS // N_CORES             # 512 output rows per core
EPS = 1e-6
F32 = mybir.dt.float32
BF16 = mybir.dt.bfloat16
AX = mybir.AxisListType
ALU = mybir.AluOpType
AF = mybir.ActivationFunctionType

KB = D // 128                     # 16 contraction blocks for projections
MB = ROWS // 128                  # 32 row blocks
QKV = GS * HD + 2 * HD            # 384 projected features per core
NQK = GS + 1                      # 5 heads that get rmsnorm+rope (4 q + 1 k)
SQT = 512                         # attention query-tile width
SKT = 128                         # attention key-tile height
JQ = S // SQT                     # 4 query tiles per batch
IKB = S // SKT                    # 16 key blocks per batch


def _build():
    nc = bacc.Bacc(num_devices=N_CORES)

    xT = nc.dram_tensor("xT", [D, ROWS], BF16, kind="ExternalInput")
    wqkv = nc.dram_tensor("wqkv", [D, QKV], BF16, kind="ExternalInput")
    wo = nc.dram_tensor("wo", [H * HD, D], BF16, kind="ExternalInput")
    cs = nc.dram_tensor("cs", [S, HD], F32, kind="ExternalInput")
    sn = nc.dram_tensor("sn", [S, HD], F32, kind="ExternalInput")
    wvec = nc.dram_tensor("wvec", [1, NQK * HD], F32, kind="ExternalInput")
    maskM = nc.dram_tensor("maskM", [128, 1024], BF16, kind="ExternalInput")
    out_rows = nc.dram_tensor("out_rows", [RPC, D], F32, kind="ExternalOutput")

    with tile.TileContext(nc) as tc:
        with (
            tc.tile_pool(name="const", bufs=1) as const,
            tc.tile_pool(name="dram", bufs=1, space="DRAM") as dram,
        ):
            a2a_in01 = dram.tile([N_CORES, 2 * HD, RPC], BF16)
            a2a_out01 = dram.tile([N_CORES, 2 * HD, RPC], BF16)
            a2a_in23 = dram.tile([N_CORES, 2 * HD, RPC], BF16)
            a2a_out23 = dram.tile([N_CORES, 2 * HD, RPC], BF16)

            w_sb = const.tile([128, KB, QKV], BF16)
            nc.sync.dma_start(w_sb[:], wqkv[:].rearrange("(k p) j -> p k j", p=128))
            cos_sb = const.tile([128, S // 128, HD], F32)
            sin_sb = const.tile([128, S // 128, HD], F32)
            nc.sync.dma_start(cos_sb[:], cs[:].rearrange("(m p) d -> p m d", p=128))
            nc.sync.dma_start(sin_sb[:], sn[:].rearrange("(m p) d -> p m d", p=128))
            mask_sb = const.tile([128, 1024], BF16)
            nc.sync.dma_start(mask_sb[:], maskM[:])
            ident = const.tile([128, 128], F32)
            make_identity(nc, ident)
            wv1 = const.tile([1, NQK * HD], F32)
            nc.sync.dma_start(wv1[:], wvec[:])
            wv_sb = const.tile([128, NQK * HD], F32)
            nc.gpsimd.partition_broadcast(wv_sb[:], wv1[0:1, :])
            eps_sb = const.tile([128, 1], F32)
            nc.vector.memset(eps_sb[:], EPS)

            # persistent activations, split per batch so attention on
            # batch 0 can overlap projection of batch 1
            qT_a = [const.tile([128, S], BF16, name=f"qT_a{bb}") for bb in range(B)]
            qT_b = [const.tile([128, S], BF16, name=f"qT_b{bb}") for bb in range(B)]
            kT_t = [const.tile([128, S], BF16, name=f"kT{bb}") for bb in range(B)]
            v1_t = [const.tile([128, S // 128, 2 * HD], BF16, name=f"v1{bb}")
                    for bb in range(B)]
            for bb in range(B):
                nc.vector.memset(v1_t[bb][:], 1.0)  # cols 64:128 stay 1.0 (denominator)

            # attention PSUM pool first on the stack so it does not overlap
            # the projection pools (overlap would serialize the phases)
            with (
                tc.tile_pool(name="ps", bufs=3, space="PSUM") as pspool,
                tc.tile_pool(name="pc", bufs=2, space="PSUM") as pcpool,
                tc.tile_pool(name="xs", bufs=20) as xs,
                tc.tile_pool(name="ev", bufs=3) as ev,
                tc.tile_pool(name="ex", bufs=6) as ex,
                tc.tile_pool(name="cn", bufs=3) as cn,
                tc.tile_pool(name="cx", bufs=1) as cx,
                tc.tile_pool(name="ws", bufs=12) as ws,
                tc.tile_pool(name="ou", bufs=3) as ou,
            ):
                from contextlib import ExitStack
                pstack = ExitStack()
                ppool = pstack.enter_context(
                    tc.tile_pool(name="pp", bufs=2, space="PSUM"))
                tpool = pstack.enter_context(
                    tc.tile_pool(name="tp", bufs=1, space="PSUM"))
                for m4 in range(MB // 4):
                    xts = []
                    for k in range(KB):
                        t = xs.tile([128, 512], BF16, tag="xts")
                        nc.sync.dma_start(
                            t[:], xT[k * 128:(k + 1) * 128, m4 * 512:(m4 + 1) * 512]
                        )
                        xts.append(t)
                    for i in range(4):
                        m = m4 * 4 + i
                        pp = ppool.tile([128, QKV], F32, tag="pp")
                        for k in range(KB):
                            nc.tensor.matmul(
                                pp[:],
                                xts[k][:, i * 128:(i + 1) * 128],
                                w_sb[:, k, :],
                                start=(k == 0),
                                stop=(k == KB - 1),
                            )
                        # rmsnorm over each 64-wide head slice (q0..q3, k)
                        nqk = NQK * HD
                        sq = ev.tile([128, nqk], F32, tag="sq")
                        nc.scalar.activation(sq[:], pp[:, :nqk], AF.Square)
                        ssum = ev.tile([128, NQK], F32, tag="ssum")
                        nc.vector.tensor_reduce(
                            ssum[:], sq[:].rearrange("p (h d) -> p h d", d=HD),
                            AX.X, ALU.add,
                        )
                        srt = ev.tile([128, NQK], F32, tag="srt")
                        nc.scalar.activation(srt[:], ssum[:], AF.Sqrt,
                                             bias=eps_sb[:], scale=1.0 / HD)
                        rs = ev.tile([128, NQK], F32, tag="rs")
                        nc.vector.reciprocal(rs[:], srt[:])
                        qkn = ev.tile([128, nqk], F32, tag="qkn")
                        for h in range(NQK):
                            nc.vector.tensor_scalar_mul(
                                qkn[:, h * HD:(h + 1) * HD],
                                pp[:, h * HD:(h + 1) * HD],
                                rs[:, h:h + 1],
                            )
                        nc.vector.tensor_mul(qkn[:], qkn[:], wv_sb[:])
                        # rope (rotate-half) on all 5 heads at once
                        sm = m % (S // 128)
                        hf = HD // 2
                        qv = qkn[:].rearrange("p (h d) -> p h d", d=HD)
                        qkr = ev.tile([128, nqk], F32, tag="qkr")
                        rv = qkr[:].rearrange("p (h d) -> p h d", d=HD)
                        t1 = ev.tile([128, NQK, hf], F32, tag="t1")
                        t2 = ev.tile([128, NQK, hf], F32, tag="t2")

                        def bmul(out_ap, a_ap, trig, lo):
                            tr = trig[:, sm:sm + 1, lo * hf:(lo + 1) * hf]
                            a2, b2 = broadcast_tensor_aps(a_ap, tr)
                            nc.vector.tensor_tensor(out_ap, a2, b2, ALU.mult)

                        lo_in = qv[:, :, 0:hf]
                        hi_in = qv[:, :, hf:HD]
                        bmul(t1[:], hi_in, sin_sb, 0)        # x2 * sin_lo
                        bmul(t2[:], lo_in, sin_sb, 1)        # x1 * sin_hi
                        bmul(rv[:, :, 0:hf], lo_in, cos_sb, 0)
                        bmul(rv[:, :, hf:HD], hi_in, cos_sb, 1)
                        nc.vector.tensor_sub(rv[:, :, 0:hf], rv[:, :, 0:hf], t1[:])
                        nc.vector.tensor_add(rv[:, :, hf:HD], rv[:, :, hf:HD], t2[:])
                        # v straight from psum (no norm/rope)
                        bb, sm2 = m // (S // 128), m % (S // 128)
                        nc.vector.tensor_copy(v1_t[bb][:, sm2, 0:HD], pp[:, nqk:QKV])
                        # transposes: [seq,hd] -> [hd,seq]
                        tq1 = tpool.tile([128, 128], F32, tag="tq")
                        nc.tensor.transpose(tq1[:], qkr[:, 0:128], ident[:])
                        nc.vector.tensor_copy(qT_a[bb][:, sm2 * 128:(sm2 + 1) * 128], tq1[:])
                        tq2 = tpool.tile([128, 128], F32, tag="tq")
                        nc.tensor.transpose(tq2[:], qkr[:, 128:256], ident[:])
                        nc.vector.tensor_copy(qT_b[bb][:, sm2 * 128:(sm2 + 1) * 128], tq2[:])
                        kst = ev.tile([128, 128], F32, tag="kst")
                        nc.vector.tensor_copy(kst[:, 0:64], qkr[:, 256:320])
                        nc.vector.tensor_copy(kst[:, 64:128], qkr[:, 256:320])
                        tq3 = tpool.tile([128, 128], F32, tag="tq")
                        nc.tensor.transpose(tq3[:], kst[:], ident[:])
                        nc.vector.tensor_copy(kT_t[bb][:, sm2 * 128:(sm2 + 1) * 128], tq3[:])

                pstack.close()  # free proj PSUM banks for the out-proj pool
                popool_cm = tc.tile_pool(name="po", bufs=3, space="PSUM")
                popool = popool_cm.__enter__()
                o1p_cm = tc.tile_pool(name="o1p", bufs=16)
                o1p = o1p_cm.__enter__()

                # ------------ phase 2: attention (overlaps phase 1)
                for b in range(B):
                    for h in range(GS):
                        qT_t = qT_a[b] if h < 2 else qT_b[b]
                        hp = (h % 2) * 64
                        for jq in range(JQ):
                            q_rhs = qT_t[hp:hp + 64,
                                         jq * SQT:(jq + 1) * SQT]
                            pctx = pcpool.tile([2 * HD, SQT], F32, tag="pctx")
                            nkb = (jq + 1) * (SQT // SKT)
                            # chunks of 4 sk-blocks: emit 4 QKs, 4 exps, then
                            # 4 PVs so PE gets dense multi-matmul runs
                            for c0 in range(0, nkb, 4):
                                iks = range(c0, min(c0 + 4, nkb))
                                ess = []
                                for ik in iks:
                                    pss = pspool.tile([128, SQT], F32, tag="pss",
                                                      name=f"pss{ik}")
                                    k_lhs = kT_t[b][hp:hp + 64,
                                                  ik * SKT:(ik + 1) * SKT]
                                    nc.tensor.matmul(pss[:], k_lhs, q_rhs,
                                                     start=True, stop=True)
                                    es = ex.tile([128, SQT], BF16, tag="es",
                                                 name=f"es{ik}")
                                    nc.scalar.activation(es[:], pss[:], AF.Exp,
                                                         scale=1.0 / np.sqrt(HD))
                                    dd = ik * SKT - jq * SQT
                                    if dd >= 0:
                                        off = 512 - dd
                                        nc.vector.tensor_mul(
                                            es[:], es[:], mask_sb[:, off:off + SQT])
                                    ess.append(es)
                                for ik, es in zip(iks, ess):
                                    nc.tensor.matmul(
                                        pctx[:],
                                        v1_t[b][:, ik, :],
                                        es[:],
                                        start=(ik == 0),
                                        stop=(ik == nkb - 1),
                                    )
                            rinv = cn.tile([64, SQT], F32, tag="rinv")
                            nc.vector.reciprocal(rinv[:], pctx[HD:2 * HD, :])
                            ctxn = cn.tile([64, SQT], BF16, tag="ctxn")
                            nc.vector.tensor_mul(ctxn[:], pctx[0:HD, :], rinv[:])
                            a2a_dst = a2a_in01 if h < 2 else a2a_in23
                            hh = h % 2
                            nc.sync.dma_start(
                                a2a_dst[b * JQ + jq, hh * HD:(hh + 1) * HD, :],
                                ctxn[:])

                # ------------ phase 3: split all-to-all + row-parallel out-proj
                # CC#1 (heads 0,1) fires while attention on heads 2,3 still
                # runs; the even half of the out-projection overlaps too.
                nc.gpsimd.collective_compute(
                    "AllToAll", ALU.bypass,
                    replica_groups=[list(range(N_CORES))],
                    ins=[a2a_in01.opt()], outs=[a2a_out01.opt()])
                nc.gpsimd.collective_compute(
                    "AllToAll", ALU.bypass,
                    replica_groups=[list(range(N_CORES))],
                    ins=[a2a_in23.opt()], outs=[a2a_out23.opt()])
                flat01 = a2a_out01[:].rearrange("g f r -> (g f) r")
                flat23 = a2a_out23[:].rearrange("g f r -> (g f) r")
                cxt = {}
                for k in range(KB):
                    fl = flat01 if k % 2 == 0 else flat23
                    g = k // 2
                    t = cx.tile([128, RPC], BF16, tag=f"cx{k}", name=f"cx{k}")
                    nc.sync.dma_start(t[:], fl[g * 128:(g + 1) * 128, :])
                    cxt[k] = t
                NH = KB // 2
                o1s = {}
                for n in range(D // 512):
                    wts = []
                    for g in range(NH):
                        wt = ws.tile([128, 512], BF16, tag="wt", name=f"wte{n}_{g}")
                        k = 2 * g
                        nc.sync.dma_start(
                            wt[:], wo[k * 128:(k + 1) * 128, n * 512:(n + 1) * 512])
                        wts.append(wt)
                    for mi in range(4):
                        p1 = popool.tile([128, 512], F32, tag="po",
                                         name=f"p1_{n}_{mi}")
                        for g in range(NH):
                            nc.tensor.matmul(
                                p1[:], cxt[2 * g][:, mi * 128:(mi + 1) * 128],
                                wts[g][:], start=(g == 0), stop=(g == NH - 1))
                        o1 = o1p.tile([128, 512], F32, tag="o1", name=f"o1_{n}_{mi}")
                        nc.vector.tensor_copy(o1[:], p1[:])
                        o1s[(n, mi)] = o1
                for n in range(D // 512):
                    wts = []
                    for g in range(NH):
                        wt = ws.tile([128, 512], BF16, tag="wt", name=f"wto{n}_{g}")
                        k = 2 * g + 1
                        nc.sync.dma_start(
                            wt[:], wo[k * 128:(k + 1) * 128, n * 512:(n + 1) * 512])
                        wts.append(wt)
                    for mi in range(4):
                        p2 = popool.tile([128, 512], F32, tag="po",
                                         name=f"p2_{n}_{mi}")
                        for g in range(NH):
                            nc.tensor.matmul(
                                p2[:], cxt[2 * g + 1][:, mi * 128:(mi + 1) * 128],
                                wts[g][:], start=(g == 0), stop=(g == NH - 1))
                        ot = ou.tile([128, 512], F32, tag="ot", name=f"ot{n}_{mi}")
                        nc.vector.tensor_add(ot[:], p2[:], o1s[(n, mi)][:])
                        nc.sync.dma_start(
                            out_rows[mi * 128:(mi + 1) * 128,
                                     n * 512:(n + 1) * 512],
                            ot[:])
                popool_cm.__exit__(None, None, None)
                o1p_cm.__exit__(None, None, None)

    nc.finalize()
    return nc


_NC_CACHE = None


def _get_nc():
    global _NC_CACHE
    if _NC_CACHE is None:
        _NC_CACHE = _build()
    return _NC_CACHE


def _host_prep(x, cos, sin, Wq, Wk, Wv, Wo, q_norm_w, k_norm_w):
    import ml_dtypes
    BF = ml_dtypes.bfloat16
    xT = np.ascontiguousarray(
        np.asarray(x, np.float32).transpose(2, 0, 1).reshape(D, ROWS).astype(BF))
    f = np.arange(1024)[None, :]
    p = np.arange(128)[:, None]
    maskM = (p + 512 <= f).astype(BF)
    wvec = np.concatenate(
        [np.tile(np.asarray(q_norm_w, np.float32), GS),
         np.asarray(k_norm_w, np.float32)]).reshape(1, NQK * HD)
    base = dict(
        cs=np.ascontiguousarray(np.asarray(cos, np.float32)),
        sn=np.ascontiguousarray(np.asarray(sin, np.float32)),
        maskM=maskM,
        wvec=np.ascontiguousarray(wvec),
        xT=xT,
    )
    wo_c = np.ascontiguousarray(np.asarray(Wo, np.float32).astype(BF))
    in_maps = []
    for c in range(N_CORES):
        wqkv = np.concatenate(
            [np.asarray(Wq, np.float32)[:, c * GS * HD:(c + 1) * GS * HD],
             np.asarray(Wk, np.float32)[:, c * HD:(c + 1) * HD],
             np.asarray(Wv, np.float32)[:, c * HD:(c + 1) * HD]], axis=1)
        in_maps.append(dict(base, wqkv=np.ascontiguousarray(wqkv.astype(BF)),
                            wo=wo_c))
    return in_maps


def kernel(x, mask, cos, sin, Wq, Wk, Wv, Wo, q_norm_w, k_norm_w, _trace=False,
           **kw):
    nc = _get_nc()
    in_maps = _host_prep(x, cos, sin, Wq, Wk, Wv, Wo, q_norm_w, k_norm_w)
    res = run_bass_kernel_spmd(nc, in_maps, list(range(N_CORES)), trace=_trace,
                               **kw)
    out = np.concatenate([res.results[c]["out_rows"] for c in range(N_CORES)],
                         axis=0)
    out = out.reshape(B, S, D).astype(np.float32)
    if _trace:
        return out, res
    return out



# revision 10
# speedup vs baseline: 1.1746x; 1.1746x over previous
"""GQA attention kernel for 8 trn2 NeuronCores.

Sharding: tensor-parallel over the 8 KV groups (1 group = 4 Q heads per
core, both batch elements), then AllToAlls reshard the per-core context
into row-shards [2048 feat, 512 rows] so the output projection runs
row-parallel with no reduction.

Key perf structure (v2):
- Single ACT table set (Ln+Exp): rmsnorm rsqrt = exp(-0.5*ln(ms)), no
  Sqrt/Square activations -> no table thrash with the softmax Exp.
- k-head rmsnorm scale is deferred into the softmax exp's per-partition
  scale AP (scores partition dim = k positions), saving a whole multiply.
- norm weights folded into the rope trig tables on the host.
- QK head pairs run concurrently on the PE via tile_position row-tiling
  (K=64 each, rows 0-63 / 64-127).
- exp over [128, 2*512] PSUM chunks (both heads of one k-block) to
  amortize ACT overhead.
- causal masking: triangular [128,128] multiply only on diagonal blocks;
  PV matmuls skip fully-masked columns.
- softmax denominators via ones-columns in the PV matmul; division uses
  reciprocal_approx_fast (bf16-accurate) instead of iterative divide.
- transposes via DMA xbar (SBUF->SBUF), freeing PE/PSUM.
- bf16 everywhere off the PE accumulators; output written bf16.

Shapes (hardcoded): B=2, S=2048, D=2048, H=32, G=8, HD=64.
"""

import math
import numpy as np
import concourse.bass as bass
import concourse.mybir as mybir
import concourse.tile as tile
from concourse import bacc
from concourse.bass import broadcast_tensor_aps
from concourse.bass_utils import run_bass_kernel_spmd

N_CORES = 8
B, S, D = 2, 2048, 2048
H, G, HD = 32, 8, 64
GS = H // G                       # 4 q heads per kv group
ROWS = B * S                      # 4096 flattened (b, s) rows
RPC = ROWS // N_CORES             # 512 output rows per core
EPS = 1e-6
F32 = mybir.dt.float32
BF16 = mybir.dt.bfloat16
AX = mybir.AxisListType
ALU = mybir.AluOpType
AF = mybir.ActivationFunctionType

KB = D // 128                     # 16 contraction blocks for projections
MB = ROWS // 128                  # 32 row blocks
SB = S // 128                     # 16 row blocks per batch
QKV = GS * HD + 2 * HD            # 384 projected features per core
NQK = GS + 1                      # 5 heads that get rmsnorm+rope (4 q + 1 k)
SQT = 512                         # attention query-tile width
SKT = 128                         # attention key-tile height
JQ = S // SQT                     # 4 query tiles per batch
HF = HD // 2


def _build():
    nc = bacc.Bacc(num_devices=N_CORES)

    xT = nc.dram_tensor("xT", [D, ROWS], BF16, kind="ExternalInput")
    wqkv = nc.dram_tensor("wqkv", [D, QKV], BF16, kind="ExternalInput")
    wo = nc.dram_tensor("wo", [H * HD, D], BF16, kind="ExternalInput")
    cs5 = nc.dram_tensor("cs5", [S, NQK * HD], BF16, kind="ExternalInput")
    sn5 = nc.dram_tensor("sn5", [S, NQK * HD], BF16, kind="ExternalInput")
    triM = nc.dram_tensor("triM", [128, 128], BF16, kind="ExternalInput")
    out_rows = nc.dram_tensor("out_rows", [RPC, D], BF16, kind="ExternalOutput")

    with tile.TileContext(nc) as tc:
        with (
            tc.tile_pool(name="const", bufs=1) as const,
            tc.tile_pool(name="dram", bufs=1, space="DRAM") as dram,
        ):
            a2a_in = [dram.tile([N_CORES, 2 * HD, RPC], BF16, name=f"a2ai{p}")
                      for p in range(2)]
            a2a_out = [dram.tile([N_CORES, 2 * HD, RPC], BF16, name=f"a2ao{p}")
                       for p in range(2)]

            w_sb = const.tile([128, KB, QKV], BF16)
            nc.sync.dma_start(w_sb[:], wqkv[:].rearrange("(k p) j -> p k j", p=128))
            cos_sb = const.tile([128, SB, NQK, HD], BF16)
            sin_sb = const.tile([128, SB, NQK, HD], BF16)
            nc.sync.dma_start(
                cos_sb[:], cs5[:].rearrange("(m p) (h d) -> p m h d", p=128, d=HD))
            nc.sync.dma_start(
                sin_sb[:], sn5[:].rearrange("(m p) (h d) -> p m h d", p=128, d=HD))
            tri_sb = const.tile([128, 128], BF16)
            nc.sync.dma_start(tri_sb[:], triM[:])
            eps_sb = const.tile([128, 1], F32)
            nc.vector.memset(eps_sb[:], EPS)
            ln8_sb = const.tile([128, 1], F32)
            nc.vector.memset(ln8_sb[:], math.log(1.0 / math.sqrt(HD)))

            # persistent activations (transposed q/k, v, k-norm scales)
            qT_a = [const.tile([128, S], BF16, name=f"qT_a{bb}") for bb in range(B)]
            qT_b = [const.tile([128, S], BF16, name=f"qT_b{bb}") for bb in range(B)]
            kT_t = [const.tile([128, S], BF16, name=f"kT{bb}") for bb in range(B)]
            v1_t = [const.tile([128, SB, 2 * HD], BF16, name=f"v1{bb}")
                    for bb in range(B)]
            rsk = [const.tile([128, SB], F32, name=f"rsk{bb}") for bb in range(B)]
            for bb in range(B):
                nc.vector.memset(v1_t[bb][:], 1.0)  # cols 64:128 stay 1.0 (denom)

            with (
                tc.tile_pool(name="ps", bufs=2, space="PSUM") as pspool,
                tc.tile_pool(name="pc", bufs=2, space="PSUM") as pcpool,
                tc.tile_pool(name="xs", bufs=2) as xs,
                tc.tile_pool(name="ev", bufs=3) as ev,
                tc.tile_pool(name="qk", bufs=4) as qk,
                tc.tile_pool(name="ex", bufs=4) as ex,
                tc.tile_pool(name="cn", bufs=3) as cn,
                tc.tile_pool(name="cx", bufs=1) as cx,
                tc.tile_pool(name="ws", bufs=5) as ws,
                tc.tile_pool(name="o1", bufs=1) as o1p,
                tc.tile_pool(name="ou", bufs=3) as ou,
            ):
                from contextlib import ExitStack
                pstack = ExitStack()
                ppool = pstack.enter_context(
                    tc.tile_pool(name="pp", bufs=2, space="PSUM"))

                # ---------------- phase 1: qkv projection + norm + rope
                for m4 in range(MB // 4):
                    xf = xs.tile([128, KB, 512], BF16, tag="xf")
                    nc.sync.dma_start(
                        xf[:],
                        xT[:, m4 * 512:(m4 + 1) * 512].rearrange(
                            "(k p) m -> p k m", p=128))
                    for i in range(4):
                        m = m4 * 4 + i
                        bb, sm = m // SB, m % SB
                        pp = ppool.tile([128, QKV], F32, tag="pp")
                        for k in range(KB):
                            nc.tensor.matmul(
                                pp[:],
                                xf[:, k, i * 128:(i + 1) * 128],
                                w_sb[:, k, :],
                                start=(k == 0),
                                stop=(k == KB - 1),
                            )
                        nqk = NQK * HD
                        # sum of squares per 64-wide head slice
                        sq = ev.tile([128, nqk], BF16, tag="sq")
                        nc.scalar.activation(sq[:], pp[:, :nqk], AF.Square)
                        ssum = ev.tile([128, NQK], F32, tag="ssum")
                        nc.vector.tensor_reduce(
                            ssum[:], sq[:].rearrange("p (h d) -> p h d", d=HD),
                            AX.X, ALU.add,
                        )
                        # rsqrt via ln/exp (stays in the Exp table set)
                        lnm = ev.tile([128, NQK], F32, tag="lnm")
                        nc.scalar.activation(lnm[:], ssum[:], AF.Ln,
                                             bias=eps_sb[:], scale=1.0 / HD)
                        rs = ev.tile([128, GS], F32, tag="rs")
                        nc.scalar.activation(rs[:], lnm[:, 0:GS], AF.Exp,
                                             scale=-0.5)
                        # k-head scale deferred to softmax: rsk = ms_k^-.5 / 8
                        nc.scalar.activation(rsk[bb][:, sm:sm + 1],
                                             lnm[:, GS:NQK], AF.Exp,
                                             bias=ln8_sb[:], scale=-0.5)
                        # q normalized (bf16); k copied raw; v copied raw
                        qkn = qk.tile([128, nqk], BF16, tag="qkn")
                        qv = pp[:, 0:GS * HD].rearrange("p (h d) -> p h d", d=HD)
                        rv = rs[:].rearrange("p (h o) -> p h o", o=1)
                        a2, b2 = broadcast_tensor_aps(qv, rv)
                        nc.vector.tensor_tensor(
                            qkn[:, 0:GS * HD].rearrange("p (h d) -> p h d", d=HD),
                            a2, b2, ALU.mult)
                        nc.scalar.activation(qkn[:, GS * HD:nqk],
                                             pp[:, GS * HD:nqk], AF.Copy)
                        # v goes in cols 64:128; cols 0:64 stay 1.0 so the
                        # PV matmul's denominator lands at partitions 0:63
                        nc.scalar.activation(v1_t[bb][:, sm, HD:2 * HD],
                                             pp[:, nqk:QKV], AF.Copy)
                        # rope (rotate-half), w folded into trig tables
                        qkr = qk.tile([128, nqk], BF16, tag="qkr")
                        qn3 = qkn[:].rearrange("p (h d) -> p h d", d=HD)
                        qr3 = qkr[:].rearrange("p (h d) -> p h d", d=HD)
                        cs3 = cos_sb[:, sm]
                        sn3 = sin_sb[:, sm]
                        t1 = ev.tile([128, NQK, HF], BF16, tag="t1")
                        t2 = ev.tile([128, NQK, HF], BF16, tag="t2")
                        nc.vector.tensor_mul(t1[:], qn3[:, :, HF:HD],
                                             sn3[:, :, 0:HF])
                        nc.vector.tensor_mul(t2[:], qn3[:, :, 0:HF],
                                             sn3[:, :, HF:HD])
                        nc.vector.tensor_mul(qr3[:, :, 0:HF], qn3[:, :, 0:HF],
                                             cs3[:, :, 0:HF])
                        nc.vector.tensor_mul(qr3[:, :, HF:HD], qn3[:, :, HF:HD],
                                             cs3[:, :, HF:HD])
                        nc.vector.tensor_sub(qr3[:, :, 0:HF], qr3[:, :, 0:HF],
                                             t1[:])
                        nc.vector.tensor_add(qr3[:, :, HF:HD], qr3[:, :, HF:HD],
                                             t2[:])
                        # k duplicated to 128 partitions for head-pair packing
                        kst = ev.tile([128, 128], BF16, tag="kst")
                        nc.vector.tensor_copy(kst[:, 0:HD], qkr[:, GS * HD:nqk])
                        nc.vector.tensor_copy(kst[:, HD:128], qkr[:, GS * HD:nqk])
                        # transposes via DMA xbar: [seq,hd] -> [hd,seq]
                        nc.sync.dma_start_transpose(
                            qT_a[bb][:, sm * 128:(sm + 1) * 128], qkr[:, 0:128])
                        nc.sync.dma_start_transpose(
                            qT_b[bb][:, sm * 128:(sm + 1) * 128], qkr[:, 128:256])
                        nc.sync.dma_start_transpose(
                            kT_t[bb][:, sm * 128:(sm + 1) * 128], kst[:])

                pstack.close()  # free proj PSUM for the out-proj pool
                popool_cm = tc.tile_pool(name="po", bufs=2, space="PSUM")
                popool = popool_cm.__enter__()

                # ---------------- phase 2: attention, head-pair concurrent
                o1s = {}

                def outproj_half(half):
                    cxa = cx.tile([128, G, RPC], BF16, tag=f"cx{half}",
                                  name=f"cx{half}")
                    nc.sync.dma_start(
                        cxa[:],
                        a2a_out[half][:].rearrange("g p r -> p g r"))
                    wov = wo[:].rearrange("(g t p) n -> p t g n", g=G, t=2)
                    for n in range(D // 512):
                        wt = ws.tile([128, G, 512], BF16, tag="wt",
                                     name=f"wt{half}_{n}")
                        nc.sync.dma_start(
                            wt[:], wov[:, half, :, n * 512:(n + 1) * 512])
                        for mi in range(4):
                            po = popool.tile([128, 512], F32, tag="po",
                                             name=f"po{half}_{n}_{mi}")
                            for g in range(G):
                                nc.tensor.matmul(
                                    po[:], cxa[:, g, mi * 128:(mi + 1) * 128],
                                    wt[:, g, :], start=(g == 0),
                                    stop=(g == G - 1))
                            if half == 0:
                                t = o1p.tile([128, 512], BF16,
                                             tag=f"o1_{n}_{mi}",
                                             name=f"o1_{n}_{mi}")
                                nc.vector.tensor_copy(t[:], po[:])
                                o1s[(n, mi)] = t
                            else:
                                ot = ou.tile([128, 512], BF16, tag="ot",
                                             name=f"ot{n}_{mi}")
                                nc.vector.tensor_add(ot[:], po[:],
                                                     o1s[(n, mi)][:])
                                nc.sync.dma_start(
                                    out_rows[mi * 128:(mi + 1) * 128,
                                             n * 512:(n + 1) * 512],
                                    ot[:])

                for pair in range(2):
                    for b in range(B):
                        qT_t = qT_a[b] if pair == 0 else qT_b[b]
                        for jq in range(JQ):
                            q_rhs = qT_t[:, jq * SQT:(jq + 1) * SQT]
                            pc0 = pcpool.tile([2 * HD, SQT], F32, tag="pc",
                                              name=f"pc0_{pair}_{b}_{jq}")
                            pc1 = pcpool.tile([2 * HD, SQT], F32, tag="pc",
                                              name=f"pc1_{pair}_{b}_{jq}")
                            nkb = (jq + 1) * (SQT // SKT)
                            for ik in range(nkb):
                                pss = pspool.tile([128, 2, SQT], F32, tag="pss")
                                ksl = kT_t[b][:, ik * SKT:(ik + 1) * SKT]
                                nc.tensor.matmul(pss[:, 0, :], ksl[0:HD, :],
                                                 q_rhs[0:HD, :],
                                                 start=True, stop=True)
                                nc.tensor.matmul(pss[:, 1, :], ksl[HD:128, :],
                                                 q_rhs[HD:128, :],
                                                 start=True, stop=True)
                                es = ex.tile([128, 2, SQT], BF16, tag="es")
                                nc.scalar.activation(
                                    es[:], pss[:], AF.Exp,
                                    scale=rsk[b][:, ik:ik + 1])
                                dd = ik * SKT - jq * SQT
                                lo = 0
                                if dd >= 0:
                                    lo = dd
                                    nc.vector.tensor_mul(
                                        es[:, 0, dd:dd + 128],
                                        es[:, 0, dd:dd + 128], tri_sb[:])
                                    nc.vector.tensor_mul(
                                        es[:, 1, dd:dd + 128],
                                        es[:, 1, dd:dd + 128], tri_sb[:])
                                nc.tensor.matmul(
                                    pc0[:, lo:SQT], v1_t[b][:, ik, :],
                                    es[:, 0, lo:SQT],
                                    start=(ik == 0), stop=(ik == nkb - 1))
                                nc.tensor.matmul(
                                    pc1[:, lo:SQT], v1_t[b][:, ik, :],
                                    es[:, 1, lo:SQT],
                                    start=(ik == 0), stop=(ik == nkb - 1))
                            # divide by denominators, ship bf16 context
                            ctxn = cn.tile([HD, 2, SQT], BF16, tag="ctxn")
                            for hh, pcx in ((0, pc0), (1, pc1)):
                                rinv = cn.tile([HD, SQT], F32, tag="rinv")
                                nc.vector.reciprocal_approx_fast(
                                    rinv[:], pcx[0:HD, :])
                                nc.vector.tensor_mul(ctxn[:, hh, :],
                                                     pcx[HD:2 * HD, :], rinv[:])
                            nc.sync.dma_start(
                                a2a_in[pair][b * JQ + jq].rearrange(
                                    "(g f) r -> f g r", g=2),
                                ctxn[:])
                    nc.gpsimd.collective_compute(
                        "AllToAll", ALU.bypass,
                        replica_groups=[list(range(N_CORES))],
                        ins=[a2a_in[pair].opt()], outs=[a2a_out[pair].opt()])
                    outproj_half(pair)

                popool_cm.__exit__(None, None, None)

    nc.finalize()
    return nc


_NC_CACHE = None


def _get_nc():
    global _NC_CACHE
    if _NC_CACHE is None:
        _NC_CACHE = _build()
    return _NC_CACHE


def _host_prep(x, cos, sin, Wq, Wk, Wv, Wo, q_norm_w, k_norm_w):
    import ml_dtypes
    BF = ml_dtypes.bfloat16
    xT = np.ascontiguousarray(
        np.asarray(x, np.float32).transpose(2, 0, 1).reshape(D, ROWS).astype(BF))
    cos = np.asarray(cos, np.float32)
    sin = np.asarray(sin, np.float32)
    wq = np.asarray(q_norm_w, np.float32)
    wk = np.asarray(k_norm_w, np.float32)
    wrot = lambda w: np.concatenate([w[HF:], w[:HF]])
    # per-head trig tables with norm weights folded in:
    # out_d = yhat_d*(cos_d*w_d) +- yhat_{d-+32}*(sin_d*w_{d-+32})
    cs_list = [cos * wq[None, :]] * GS + [cos * wk[None, :]]
    sn_list = [sin * wrot(wq)[None, :]] * GS + [sin * wrot(wk)[None, :]]
    cs5 = np.stack(cs_list, axis=1).reshape(S, NQK * HD).astype(BF)
    sn5 = np.stack(sn_list, axis=1).reshape(S, NQK * HD).astype(BF)
    p = np.arange(128)[:, None]
    f = np.arange(128)[None, :]
    triM = (f >= p).astype(BF)
    base = dict(cs5=np.ascontiguousarray(cs5), sn5=np.ascontiguousarray(sn5),
                triM=np.ascontiguousarray(triM), xT=xT)
    wo_c = np.ascontiguousarray(np.asarray(Wo, np.float32).astype(BF))
    in_maps = []
    for c in range(N_CORES):
        wqkv = np.concatenate(
            [np.asarray(Wq, np.float32)[:, c * GS * HD:(c + 1) * GS * HD],
             np.asarray(Wk, np.float32)[:, c * HD:(c + 1) * HD],
             np.asarray(Wv, np.float32)[:, c * HD:(c + 1) * HD]], axis=1)
        in_maps.append(dict(base, wqkv=np.ascontiguousarray(wqkv.astype(BF)),
                            wo=wo_c))
    return in_maps


def kernel(x, mask, cos, sin, Wq, Wk, Wv, Wo, q_norm_w, k_norm_w, _trace=False,
           **kw):
    nc = _get_nc()
    in_maps = _host_prep(x, cos, sin, Wq, Wk, Wv, Wo, q_norm_w, k_norm_w)
    res = run_bass_kernel_spmd(nc, in_maps, list(range(N_CORES)), trace=_trace,
                               **kw)
    out = np.concatenate([np.asarray(res.results[c]["out_rows"],
                                     dtype=np.float32)
                          for c in range(N_CORES)], axis=0)
    out = out.reshape(B, S, D)
    if _trace:
        return out, res
    return out
